# revision 22
# baseline (speedup 1.0000x reference)
"""Trainium2 Bass kernel for nn_Pooling_block (B=128, N=785, C=384, pp=2).

Pure data-parallel over batch: 16 batches per core x 8 NeuronCores.

v2 design notes (DMA-balance + instruction-count focused):
  - All bulk transfers are single large dma_starts so descriptors spread over
    all 16 SDMA engines: x gather (784 desc, SWDGE fp16-cast), edge fold
    (768 desc, HWDGE via sync), out write (196 desc, SWDGE).
  - x nodes arrive as one patch-gathered fp16 tile G[98, 2half, 4slot, C];
    slot s=2q+p holds node row 28p+q of each 2x2 patch.
  - edge loaded f32, partition-summed with f32r ones-matmuls (bitcast, no
    cast pass); node sums from the fp16 pair-sum tiles A.
  - ci = (sig(edge_mean)+sig(node_mean)) @ W_lin.T via f32r matmuls.
  - scores: DVE STT fp16 (2x mode) against PE-broadcast ci; sigmoid on ACT.
  - pooled[hh] = A_0*T_0 + A_1*T_1 in fp16; PE fp16 transposes -> a_cm.
  - final out = a_cm.T @ W_out_cls.T in fp16 (PE internally bf16; error
    budget ~2e-3 << 2e-2 gate). cls row handled once per core for all 16
    batches.
"""
import os
import sys

sys.path.insert(0, "/opt/trn_rl_repo")

import numpy as np

import concourse.bass as bass
import concourse.tile as tile
from concourse import bacc, mybir
from concourse.bass_utils import run_bass_kernel_spmd

B, N, C = 128, 785, 384
HW = N - 1          # 784
H = 28              # grid side
HP = 14             # pooled grid side
NPATCH = HP * HP    # 196
NB = 16             # batches per core
NCORES = 8
NOUT = 1 + NPATCH   # 197
CO = 2 * C          # 768

F32 = mybir.dt.float32
F32R = mybir.dt.float32r
BF16 = mybir.dt.bfloat16
FP16 = mybir.dt.float16
ADD = mybir.AluOpType.add
MUL = mybir.AluOpType.mult
SIG = mybir.ActivationFunctionType.Sigmoid


def build_program(w_slot):
    """w_slot = per-slot scalar weights (w[0,s%2,s//2] for s in 0..3) when the
    per-patch weights are channel-uniform, else None (general path)."""
    nc = bacc.Bacc(None, target_bir_lowering=False, debug=False,
                   dynamic_dma_scratch_size=65536)

    x_d = nc.declare_dram_parameter("x", [NB, N, C], F32, isOutput=False)
    e_d = nc.declare_dram_parameter("edge", [NB, N, C], F32, isOutput=False)
    wlt_d = nc.declare_dram_parameter("wlt", [C, C], F32, isOutput=False)
    wct_d = nc.declare_dram_parameter("wct16", [C, CO], FP16, isOutput=False)
    id_d = nc.declare_dram_parameter("ident16", [98, 98], FP16, isOutput=False)
    clsc_d = nc.declare_dram_parameter("cls_cm", [128, 3, NB], FP16, isOutput=False)
    if w_slot is None:
        wqr_d = nc.declare_dram_parameter("wqr", [4, 128, C], FP16, isOutput=False)
    out_d = nc.declare_dram_parameter("out", [NB, NOUT, CO], F32, isOutput=True)

    # gathered view of x nodes: node n = 392*hh + 56i + 28p + 2j + q
    # -> partition (i j) [98], free (hh, q, p, c); slot s=2q+p within half hh
    x_g = [
        x_d[b, 1:N, :].rearrange(
            "(hh i p j q) c -> hh p i j q c", hh=2, i=7, p=2, j=HP, q=2
        )
        for b in range(NB)
    ]
    # edge fold: partition p holds rows {p, 128+p, ..., 640+p}
    e_f = [
        e_d[b, 0:768, :].rearrange("(k p) c -> p k c", p=128) for b in range(NB)
    ]
    # out rows 1..196: partition p holds rows 1+p and 1+98+p
    out_v = [
        out_d[b, 1:NOUT, :].rearrange("(k p) co -> p k co", p=98)
        for b in range(NB)
    ]

    with tile.TileContext(nc) as tc:
        with (
            tc.tile_pool(name="const", bufs=1) as cpool,
            tc.tile_pool(name="gx", bufs=5) as gxp,
            tc.tile_pool(name="ed", bufs=6) as edp,
            tc.tile_pool(name="work", bufs=4) as wk,
            tc.tile_pool(name="small", bufs=4) as sm,
            tc.tile_pool(name="acm", bufs=3) as acmp,
            tc.tile_pool(name="ost", bufs=2) as ostp,
            tc.tile_pool(name="psA", bufs=1, space="PSUM") as psA,
            tc.tile_pool(name="psB", bufs=2, space="PSUM") as psB,
        ):
            # ---- constants ----
            ones_f = cpool.tile([128, 1], F32)
            nc.vector.memset(ones_f[:], 1.0)
            ones_r = cpool.tile([128, 1], F32R)
            nc.vector.tensor_copy(ones_r[:], ones_f[:])
            ones_h = cpool.tile([128, 1], FP16)
            nc.vector.memset(ones_h[:], 1.0)
            ones_bf = cpool.tile([128, 1], BF16)
            nc.vector.memset(ones_bf[:], 1.0)
            ones_f_row = cpool.tile([1, 128], F32)
            nc.vector.memset(ones_f_row[:], 1.0)
            ones_r_row = cpool.tile([1, 128], F32R)
            nc.vector.tensor_copy(ones_r_row[:], ones_f_row[:])
            one_f_11 = cpool.tile([1, 1], F32)
            nc.vector.memset(one_f_11[:], 1.0)

            ident16 = cpool.tile([98, 98], FP16)
            nc.sync.dma_start(ident16[:], id_d[:])

            wlt_r = []
            for cch in range(3):
                t = cpool.tile([128, C], F32, tag=f"wlt{cch}")
                nc.sync.dma_start(t[:], wlt_d[128 * cch : 128 * (cch + 1), :])
                tr = cpool.tile([128, C], F32R, tag=f"wltr{cch}")
                nc.vector.tensor_copy(tr[:], t[:])
                wlt_r.append(tr)
            wct16 = []
            for cch in range(3):
                t = cpool.tile([128, CO], FP16, tag=f"wct{cch}")
                nc.sync.dma_start(t[:], wct_d[128 * cch : 128 * (cch + 1), :])
                wct16.append(t)
            if w_slot is None:
                wqr_t = []
                for k in range(4):
                    t = cpool.tile([128, C], FP16, tag=f"wqr{k}")
                    nc.sync.dma_start(t[:], wqr_d[k])
                    wqr_t.append(t)

            wqr_row = None
            if w_slot is not None and len(set(w_slot)) > 1:
                wqr_row = cpool.tile([128, 8], F32)
                for k in range(8):
                    nc.vector.memset(wqr_row[:, k : k + 1], float(w_slot[k % 4]))

            cls_cm = cpool.tile([128, 3, NB], FP16)
            nc.sync.dma_start(cls_cm[:], clsc_d[:])

            # ---- cls row for all batches: out[:, 0, :] = cls @ W_out.T ----
            cls_st = cpool.tile([NB, CO], F32)
            for nh in range(2):
                pc = psB.tile([98, C], F32, tag="fo")
                for cch in range(3):
                    nc.tensor.matmul(
                        pc[0:NB, :],
                        cls_cm[:, cch, :],
                        wct16[cch][:, C * nh : C * (nh + 1)],
                        start=(cch == 0), stop=(cch == 2),
                    )
                nc.scalar.copy(cls_st[:, C * nh : C * (nh + 1)], pc[0:NB, :])
            nc.sync.dma_start(out_d[:, 0, :], cls_st[:])

            # ---- per-batch pipeline ----
            for b in range(NB):
                # -- loads --
                g = gxp.tile([98, 2, 4, C], FP16, tag="g")
                for hh in range(2):
                    for p in range(2):
                        nc.gpsimd.dma_start(
                            g[:, hh, 2 * p : 2 * p + 2, :], x_g[b][hh, p]
                        )

                efold = edp.tile([128, 6, C], BF16, tag="efold")
                nc.gpsimd.dma_start(efold[:], e_f[b])  # SWDGE bf16 cast, 768 desc
                etail = edp.tile([17, C], BF16, tag="etail")
                nc.gpsimd.dma_start(etail[:], e_d[b, 768:785, :])

                # -- edge sums into esns[0], node sums into esns[32] --
                esns = psA.tile([33, C], F32, tag=f"esns{b % 2}")
                es = esns[0:1, :]
                for k in range(6):
                    nc.tensor.matmul(
                        es, ones_bf[:], efold[:, k, :],
                        start=(k == 0), stop=False,
                    )
                nc.tensor.matmul(
                    es, ones_bf[0:17, :], etail[:], start=False, stop=True
                )

                # -- vertical pair sums A[hh][:, q, :] = g_s(2q) + g_s(2q+1) --
                a_t = []
                for hh in range(2):
                    at = wk.tile([98, 2, C], FP16, tag=f"a{hh}")
                    for q in range(2):
                        nc.vector.tensor_add(
                            at[:, q, :], g[:, hh, q, :], g[:, hh, 2 + q, :]
                        )
                    a_t.append(at)

                # -- node sums: ones-matmul over the 4 A slices (fp16) --
                ns = esns[32:33, :]
                first = True
                for hh in range(2):
                    for q in range(2):
                        nc.tensor.matmul(
                            ns, ones_h[0:98, :], a_t[hh][:, q, :],
                            start=first, stop=(hh == 1 and q == 1),
                        )
                        first = False

                # -- means -> sigmoid -> s row --
                se = sm.tile([1, C], F32, tag="se")
                nc.scalar.activation(se[:], esns[0:1, :], SIG, scale=1.0 / N)
                sn = sm.tile([1, C], F32, tag="sn")
                nc.scalar.activation(sn[:], esns[32:33, :], SIG, scale=1.0 / HW)
                s_row = sm.tile([1, C], F32, tag="srow")
                nc.vector.tensor_add(s_row[:], se[:], sn[:])

                # -- s row -> s col; ci row = s @ W_lin.T --
                s_colp = psA.tile([128, 3], F32, tag="smallp")
                for cch in range(3):
                    nc.tensor.matmul(
                        s_colp[:, cch : cch + 1],
                        s_row[:, 128 * cch : 128 * (cch + 1)],
                        one_f_11[:], start=True, stop=True,
                    )
                s_col = sm.tile([128, 3], F32R, tag="scol")
                nc.vector.tensor_copy(s_col[:], s_colp[:])

                cirow_p = psA.tile([1, C], F32, tag="smallp")
                for cch in range(3):
                    nc.tensor.matmul(
                        cirow_p[:], s_col[:, cch : cch + 1], wlt_r[cch][:],
                        start=(cch == 0), stop=(cch == 2),
                    )
                ci_f = sm.tile([1, C], F32R, tag="cif")
                nc.scalar.copy(ci_f[:], cirow_p[:])

                # -- broadcast ci to 128 partitions; round to fp16 on copy --
                cib_p = psA.tile([128, C], F32, tag="cibp")
                nc.tensor.matmul(
                    cib_p[:], ones_r_row[:], ci_f[:], start=True, stop=True
                )
                cib = sm.tile([128, C], FP16, tag="cib")
                nc.scalar.copy(cib[:], cib_p[:])

                # -- scores: fused mul+reduce per gather slot (fp16, DVE) --
                sacc = sm.tile([98, 8], F32, tag="sacc")
                for hh in range(2):
                    for s in range(4):
                        scr = wk.tile([98, C], FP16, tag="ttrs")
                        nc.vector.scalar_tensor_tensor(
                            scr[:], g[:, hh, s, :], 1.0, cib[0:98, :],
                            MUL, MUL, accum_out=sacc[:, 4 * hh + s : 4 * hh + s + 1],
                        )
                sigt = sm.tile([98, 8], F32, tag="sig")
                nc.scalar.activation(sigt[:], sacc[:], SIG)

                # -- pooled tiles (fp16 [98, C] per half) --
                pooled = []
                if w_slot is not None:
                    w00, w01, w10, w11 = w_slot
                    uniform_w = w00 == w01 == w10 == w11
                    # sp = sigma + 1 over all 8 slots; T[hh*2+q] = sp[4hh+2q] + sp[4hh+2q+1]
                    sp = sm.tile([98, 8], F32, tag="sp")
                    nc.vector.tensor_scalar_add(sp[:], sigt[:], 1.0)
                    if not uniform_w:
                        nc.vector.tensor_mul(sp[:], sp[:], wqr_row[0:98, :])
                    t01 = sm.tile([98, 4], F32, tag="t01")
                    nc.vector.tensor_add(t01[:], sp[:, 0:8:2], sp[:, 1:8:2])
                    if uniform_w and w00 != 1.0:
                        nc.vector.tensor_scalar_mul(t01[:], t01[:], float(w00))
                    for hh in range(2):
                        p0 = wk.tile([98, C], FP16, tag=f"p0_{hh}")
                        nc.vector.tensor_scalar_mul(
                            p0[:], a_t[hh][:, 0, :], t01[:, 2 * hh : 2 * hh + 1]
                        )
                        pl = wk.tile([98, C], FP16, tag=f"pool{hh}")
                        nc.vector.scalar_tensor_tensor(
                            pl[:], a_t[hh][:, 1, :], t01[:, 2 * hh + 1 : 2 * hh + 2],
                            p0[:], MUL, ADD,
                        )
                        pooled.append(pl)
                else:
                    sp = sm.tile([98, 8], F32, tag="sp")
                    nc.vector.tensor_scalar_add(sp[:], sigt[:], 1.0)
                    for hh in range(2):
                        acc = None
                        for s in range(4):
                            bqr = wk.tile([98, C], FP16, tag=f"bqr{hh}")
                            nc.vector.tensor_scalar_mul(
                                bqr[:], a_t[hh][:, s // 2, :],
                                sp[:, 4 * hh + s : 4 * hh + s + 1],
                            )
                            term = wk.tile([98, C], FP16, tag=f"term{hh}")
                            nc.vector.tensor_mul(term[:], bqr[:], wqr_t[s][0:98, :])
                            if acc is None:
                                acc = term
                            else:
                                nacc = wk.tile([98, C], FP16, tag=f"pacc{hh}_{s % 2}")
                                nc.vector.tensor_add(nacc[:], acc[:], term[:])
                                acc = nacc
                        pooled.append(acc)

                # -- c-major a_cm[cch] [128, 196] via PE fp16 transposes --
                a_cm = []
                for cch in range(3):
                    acm = acmp.tile([128, 2 * 98], FP16, tag=f"acm{cch}")
                    a_cm.append(acm)
                    tp = psB.tile([128, 2, 98], FP16, tag="tp")
                    for hh in range(2):
                        nc.tensor.transpose(
                            tp[:, hh, :],
                            pooled[hh][:, 128 * cch : 128 * (cch + 1)],
                            ident16[:],
                        )
                    nc.scalar.copy(acm[:], tp[:])

                # -- final matmul: out rows 1+hh*98+p --
                stile = ostp.tile([98, 2, CO], F32, tag="ost")
                for hh in range(2):
                    for nh in range(2):
                        fo = psB.tile([98, C], F32, tag="fo")
                        for cch in range(3):
                            nc.tensor.matmul(
                                fo[:],
                                a_cm[cch][:, 98 * hh : 98 * (hh + 1)],
                                wct16[cch][:, C * nh : C * (nh + 1)],
                                start=(cch == 0), stop=(cch == 2),
                            )
                        dst = stile[:, hh, C * nh : C * (nh + 1)]
                        nc.scalar.copy(dst, fo[:])
                nc.sync.dma_start(out_v[b], stile[:])  # HWDGE, 196 desc

    nc.compile()
    return nc


def make_inputs(x, edge, W_lin, W_out_cls, weights):
    """Shared host-side prep for kernel() and test harness timing runs."""
    x = np.ascontiguousarray(x, dtype=np.float32)
    edge = np.ascontiguousarray(edge, dtype=np.float32)
    wlt = np.ascontiguousarray(np.asarray(W_lin).T, dtype=np.float32)
    wct16 = np.ascontiguousarray(np.asarray(W_out_cls).T, dtype=np.float16)
    w = np.asarray(weights, dtype=np.float32)

    c_uniform = bool(np.all(w == w[0:1]))
    # slot s=2p+q; T_q = sum_r w[q,r]*(sig[slot 2q+r]+1): slot s weight w[s//2, s%2]
    w_slot = tuple(float(v) for v in w[0].reshape(4)) if c_uniform else None

    ident16 = np.eye(98, dtype=np.float16)
    in_maps = []
    for core in range(NCORES):
        sl = slice(core * NB, (core + 1) * NB)
        cls_cm = np.ascontiguousarray(
            x[sl, 0, :].T.reshape(3, 128, NB).transpose(1, 0, 2), dtype=np.float16
        )
        m = {
            "x": x[sl], "edge": edge[sl], "wlt": wlt, "wct16": wct16,
            "ident16": ident16, "cls_cm": cls_cm,
        }
        if w_slot is None:
            wqr = np.empty((4, 128, C), dtype=np.float16)
            for s in range(4):
                wqr[s] = np.broadcast_to(w[:, s // 2, s % 2], (128, C))
            m["wqr"] = wqr
        in_maps.append(m)
    return w_slot, in_maps


def kernel(x, edge, W_lin, W_out_cls, weights):
    w_slot, in_maps = make_inputs(x, edge, W_lin, W_out_cls, weights)
    nc = build_program(w_slot)
    res = run_bass_kernel_spmd(nc, in_maps, list(range(NCORES)))
    out = np.concatenate([r["out"] for r in res.results], axis=0)
    return out
